# revision 16
# baseline (speedup 1.0000x reference)
"""Trainium2 Bass kernel for nn_GrassmannNN.

Math: the reference's Z2-graded network collapses per-sample to a chain of
32x32 matmuls selected by the sample's 8 bits, so there are only 256 distinct
outputs.  Per body layer l the two transition matrices are
  M0_l = (sum_{j<16}  e0_l[j] G_l[:,j,:]) * blockdiag_mask
  M1_l = (sum_{j>=16} e1_l[j] G_l[:,j,:]) * antidiag_mask (rows 16: negated)
and x <- tanh(x @ M_b) for bit b; the head is x0 @ (head_w * blockdiag).

Device algorithm (per core, fully replicated table + 1/8 of the batch):
  1. body_w is host-packed (pure gather) into (112, 1024): rows (l, j), the
     four non-masked 16x16x16 blocks per layer as columns.  One 448KB DMA.
  2. Scale rows by e0/e1 columns (2 vector ops), then TWO float32r matmuls
     with a constant layer-indicator lhsT produce all 14 M-matrices at once.
  3. Four strided SBUF->SBUF DMAs scatter the blocks into Mpair (32, 7, 2, 32)
     = per-layer [M0 | M1] in lhsT layout (block sign folded into the psum
     copy).
  4. Doubling table build: state X_l (32, 2^l) over all bit-prefixes; per site
     two matmuls (branch 0/1 into adjacent psum column ranges) + one tanh.
     Site 7 flips orientation (lhsT = state) to land the table pattern-major
     as Th0/Th1 (128, 64) bf16 with the output zero-structure baked in.
  5. Gather: idx = pow2 @ bits (exact in bf16), one-hot via is_equal, then
     out^T (64, 1024) = Th0^T oh0 + Th1^T oh1 in 2x512 psum chunks.
  6. Output is stored transposed (64, BC); the host unshards/transposes.
"""

import numpy as np
from contextlib import ExitStack

import concourse.bass as bass
import concourse.bacc as bacc
import concourse.tile as tile
import concourse.mybir as mybir
from concourse.bass_utils import run_bass_kernel_spmd

F32 = mybir.dt.float32
F32R = mybir.dt.float32r
F16 = mybir.dt.float16
BF16 = mybir.dt.bfloat16
I32 = mybir.dt.int32
AF = mybir.ActivationFunctionType
OP = mybir.AluOpType

NCORES = 8
DEBUG = False
B = 8192
BC = B // NCORES          # 1024 samples per core

# bodypk free-dim layout
C_M0 = 0                  # cols 0:512    packed m0 blocks
C_M1 = 512                # cols 512:1024 packed m1 blocks
C_IND = 1024              # cols 1024:1031 layer indicator (112, 7)
C_MBD = 1031              # cols 1031:1063 maskbd rows 0:32
C_PW2 = 1063              # col 1063 pow2 rows 0:8
C_EMB = 1064              # cols 1064:1099: e0col | e1col | site0 [e0;e1] | head_w
C_TOT = 1099


def _host_consts():
    ind = np.zeros((112, 7), np.float32)
    for l in range(7):
        ind[16 * l:16 * l + 16, l] = 1.0
    maskbd = np.zeros((112, 32), np.float32)
    maskbd[:16, :16] = 1.0
    maskbd[16:32, 16:] = 1.0
    pw2 = np.zeros((112, 1), np.float32)
    pw2[:8, 0] = (1 << np.arange(8)).astype(np.float32)
    return ind, maskbd, pw2


def _emit(ctx: ExitStack, tc, t):
    nc = tc.nc
    body_d, datat_d, out_d = t["bodypk"], t["dataT"], t["out"]

    cpool = ctx.enter_context(tc.tile_pool(name="consts", bufs=1))
    work = ctx.enter_context(tc.tile_pool(name="work", bufs=1))
    psum = ctx.enter_context(tc.tile_pool(name="psum", bufs=1, space="PSUM"))

    # ---- input DMAs, split across both HWDGE queues ----
    bodyT = cpool.tile([112, C_TOT], F32)
    nc.sync.dma_start(bodyT[:, C_M0:C_M1], body_d.ap()[:, C_M0:C_M1])
    nc.sync.dma_start(bodyT[:, C_M1:C_IND], body_d.ap()[:, C_M1:C_IND])
    nc.scalar.dma_start(bodyT[:, C_IND:C_TOT], body_d.ap()[:, C_IND:C_TOT])
    dT32 = cpool.tile([8, BC], I32)
    nc.scalar.dma_start(dT32[:], datat_d.ap())

    # ---- small constants built on device ----
    iotaI = cpool.tile([128, 1], I32)
    nc.gpsimd.iota(iotaI[:], [[0, 1]], base=0, channel_multiplier=1)
    iotaF = cpool.tile([128, 1], F32)
    nc.vector.tensor_copy(iotaF[:], iotaI[:])
    ones8 = cpool.tile([8, 128], BF16)
    nc.vector.memset(ones8[:], 1.0)
    pow2row = cpool.tile([8, 128], BF16)
    nc.vector.tensor_scalar(pow2row[:], ones8[:], bodyT[0:8, C_PW2:C_PW2 + 1],
                            None, OP.mult)

    # ---- PE clock warmup: dummy matmuls while input DMAs land ----
    warm = cpool.tile([128, 512], BF16)
    nc.vector.memset(warm[:, 0:256], 0.0)
    pwarm = psum.tile([128, 512], F32, tag="g1")
    for _ in range(12):
        nc.tensor.matmul(pwarm[:, 0:256], warm[:, 0:128], warm[:, 0:256],
                         start=True, stop=True)

    # ---- M build: lhsT = e-values arranged block-diagonally per layer ----
    E1 = work.tile([112, 7], F32)
    nc.vector.tensor_scalar(E1[:], bodyT[:, C_IND:C_MBD], bodyT[:, C_EMB + 1:C_EMB + 2],
                            None, OP.mult)
    E0 = work.tile([112, 7], F32)
    nc.vector.tensor_scalar(E0[:], bodyT[:, C_IND:C_MBD], bodyT[:, C_EMB:C_EMB + 1],
                            None, OP.mult)
    psW1 = psum.tile([7, 512], F32, tag="w1")
    nc.tensor.matmul(psW1[:], E1[:], bodyT[:, C_M1:C_IND], start=True, stop=True)
    psW0 = psum.tile([7, 512], F32, tag="w0")
    nc.tensor.matmul(psW0[:], E0[:], bodyT[:, C_M0:C_M1], start=True, stop=True)
    W1 = work.tile([7, 512], F32)
    nc.scalar.activation(W1[:], psW1[:], AF.Copy)
    W0 = work.tile([7, 512], F32)
    nc.vector.tensor_copy(W0[:], psW0[:])
    # ---- scatter blocks into per-layer lhsT form Mpair (32, l, sel, k) ----
    # SBUF APs cannot iterate partitions non-outermost, so bounce W through
    # DRAM; the flat DRAM source AP can then iterate (i, l, k).
    dpool = ctx.enter_context(tc.tile_pool(name="dram", bufs=1, space="DRAM"))
    scr = dpool.tile([14, 512], F32)
    nc.scalar.dma_start(scr[7:14, :], W1[:])
    nc.sync.dma_start(scr[0:7, :], W0[:])
    Mpair = work.tile([32, 7, 2, 32], F32)
    nc.vector.memset(Mpair[:], 0.0)
    for sel in (1, 0):
        for blk in range(2):
            c0 = 16 * (blk ^ sel)
            src = bass.AP(scr.tensor, 3584 * sel + 256 * blk,
                          [[16, 16], [512, 7], [1, 16]])
            eng = nc.sync if blk == 0 else nc.scalar
            eng.dma_start(
                Mpair[16 * blk:16 * blk + 16, :, sel, c0:c0 + 16], src)

    # ---- one-hot path: idx = sum_j 2^j b_j, broadcast via matmul ----
    dTb = cpool.tile([8, BC], BF16)
    nc.vector.tensor_copy(dTb[:], dT32[:])
    oh0 = cpool.tile([128, BC], BF16)
    oh1 = cpool.tile([128, BC], BF16)
    pidx = []
    for c in range(2):
        pi = psum.tile([128, 512], F32, tag=f"idx{c}")
        nc.tensor.matmul(pi[:], pow2row[:], dTb[:, c * 512:(c + 1) * 512],
                         start=True, stop=True)
        pidx.append(pi)
    for c in range(2):
        sl = slice(c * 512, (c + 1) * 512)
        nc.vector.tensor_scalar(oh0[:, sl], pidx[c][:], iotaF[:, 0:1], None,
                                OP.is_equal)
        nc.vector.tensor_scalar(oh1[:, sl], pidx[c][:], 128.0, iotaF[:, 0:1],
                                OP.subtract, OP.is_equal)



    # ---- head: X0 (32,2) scaled embed, X1 = tanh(Mh^T X0) ----
    Mh = work.tile([32, 32], F32)
    nc.vector.tensor_mul(Mh[:], bodyT[0:32, C_EMB + 3:C_EMB + 35], bodyT[0:32, C_MBD:C_PW2])
    X0 = work.tile([32, 2], F32)
    nc.scalar.activation(X0[:], bodyT[0:32, C_MBD:C_MBD + 17:16], AF.Copy,
                         scale=bodyT[0:32, C_EMB + 2:C_EMB + 3])
    ps = psum.tile([32, 256], F32, tag="chain")
    nc.tensor.matmul(ps[:, 0:2], Mh[:], X0[:],
                     start=True, stop=True)
    X = work.tile([32, 2], F32, tag="X1")
    nc.scalar.activation(X[:], ps[:, 0:2], AF.Tanh)

    # ---- doubling chain, sites 1..6 ----
    for ml in range(6):
        N = 2 << ml
        ps = psum.tile([32, 256], F32, tag="chain")
        for b in range(2):
            nc.tensor.matmul(ps[:, b * N:(b + 1) * N],
                             Mpair[:, ml, b, :], X[:],
                             start=True, stop=True)
        X = work.tile([32, 2 * N], F16 if ml == 5 else F32, tag=f"X{ml + 2}")
        nc.scalar.activation(X[:], ps[:, 0:2 * N], AF.Tanh)

    # ---- site 7: land pattern-major, bake output zero structure ----
    pf = psum.tile([128, 64], F32, tag="pf")
    Th0 = work.tile([128, 4, 16], BF16)
    Th1 = work.tile([128, 4, 16], BF16)
    nc.vector.memset(Th0[:, 1:3, :], 0.0)
    nc.vector.memset(Th1[:, 1:3, :], 0.0)
    Mp6b = work.tile([32, 64], F16)
    nc.vector.tensor_copy(Mp6b[:], Mpair[:, 6, :, :])
    nc.tensor.matmul(pf[:], X[:], Mp6b[:], start=True, stop=True)
    nc.scalar.activation(Th0[:, 0::3, :], pf[:, 0:32], AF.Tanh)
    nc.scalar.activation(Th1[:, 0::3, :], pf[:, 32:64], AF.Tanh)

    # ---- gather: out^T = Th0^T oh0 + Th1^T oh1, in 512-sample chunks ----
    OUT = work.tile([64, BC], F32)
    for c in range(2):
        sl = slice(c * 512, (c + 1) * 512)
        pg = psum.tile([64, 512], F32, tag=f"g{c}")
        nc.tensor.matmul(pg[:], Th0[:].rearrange("p a b -> p (a b)"),
                         oh0[:, sl], start=True, stop=False)
        nc.tensor.matmul(pg[:], Th1[:].rearrange("p a b -> p (a b)"),
                         oh1[:, sl], start=False, stop=True)
        if c == 0:
            nc.vector.tensor_copy(OUT[:, sl], pg[:])
        else:
            nc.scalar.copy(OUT[:, sl], pg[:])
        nc.scalar.dma_start(out_d.ap()[:, sl], OUT[:, sl])
    if DEBUG:
        nc.sync.dma_start(t["dbg_w"].ap()[0:7, :], W0[:])
        nc.sync.dma_start(t["dbg_w"].ap()[7:14, :], W1[:])
        nc.sync.dma_start(t["dbg_mpair"].ap(), Mpair[:].rearrange("p a b c -> p (a b c)"))
        nc.sync.dma_start(t["dbg_th"].ap()[:, 0:64], Th0[:].rearrange("p a b -> p (a b)"))
        nc.sync.dma_start(t["dbg_th"].ap()[:, 64:128], Th1[:].rearrange("p a b -> p (a b)"))
        nc.sync.dma_start(t["dbg_oh"].ap()[:, 0:64], oh0[:, 0:64])
        nc.sync.dma_start(t["dbg_oh"].ap()[:, 64:128], oh1[:, 0:64])
        nc.sync.dma_start(t["dbg_x7"].ap(), X[:])
        nc.sync.dma_start(t["dbg_iota"].ap(), iotaF[:])
        nc.sync.dma_start(t["dbg_pow2"].ap(), pow2row[:])


def build_program():
    nc = bacc.Bacc("TRN2", target_bir_lowering=False, debug=False,
                   enable_asserts=False, num_devices=NCORES)
    t = {}
    t["bodypk"] = nc.dram_tensor("bodypk", [112, C_TOT], F32, kind="ExternalInput")
    t["dataT"] = nc.dram_tensor("dataT", [8, BC], I32, kind="ExternalInput")
    t["out"] = nc.dram_tensor("out", [64, BC], F32, kind="ExternalOutput")
    if DEBUG:
        t["dbg_w"] = nc.dram_tensor("dbg_w", [14, 512], F32, kind="ExternalOutput")
        t["dbg_mpair"] = nc.dram_tensor("dbg_mpair", [32, 448], F32, kind="ExternalOutput")
        t["dbg_th"] = nc.dram_tensor("dbg_th", [128, 128], BF16, kind="ExternalOutput")
        t["dbg_oh"] = nc.dram_tensor("dbg_oh", [128, 128], BF16, kind="ExternalOutput")
        t["dbg_x7"] = nc.dram_tensor("dbg_x7", [32, 128], F32, kind="ExternalOutput")
        t["dbg_iota"] = nc.dram_tensor("dbg_iota", [128, 1], F32, kind="ExternalOutput")
        t["dbg_pow2"] = nc.dram_tensor("dbg_pow2", [8, 128], BF16, kind="ExternalOutput")
    with tile.TileContext(nc) as tc:
        with ExitStack() as ctx:
            _emit(ctx, tc, t)
    nc.compile()
    return nc


def make_in_maps(data, embedding, head_w, body_w):
    data = np.asarray(data)
    if data.dtype == np.int64:
        d32 = data.view(np.int32).reshape(B, 16)[:, ::2]
    else:
        d32 = data.astype(np.int32, copy=False)
    embedding = np.asarray(embedding, np.float32)
    head_w = np.asarray(head_w, np.float32)
    body_w = np.asarray(body_w, np.float32)

    ind, maskbd, pw2 = _host_consts()
    bodypk = np.zeros((112, C_TOT), np.float32)
    l_, j_, blk_, i_, k_ = np.ix_(np.arange(7), np.arange(16), np.arange(2),
                                  np.arange(16), np.arange(16))
    bodypk[:, C_M0:C_M1] = body_w[l_, i_ + 16 * blk_, j_, k_ + 16 * blk_
                                  ].reshape(112, 512)
    # blk=1 half (the D block) carries the graded sign: pack negated
    m1 = body_w[l_, i_ + 16 * blk_, 16 + j_, k_ + 16 * (1 - blk_)]
    m1[:, :, 1, :, :] *= -1.0
    bodypk[:, C_M1:C_IND] = m1.reshape(112, 512)
    bodypk[:, C_IND:C_MBD] = ind
    bodypk[:, C_MBD:C_PW2] = maskbd
    bodypk[:, C_PW2:C_TOT] = pw2

    bodypk[:, C_EMB] = embedding[1:, 0, :].reshape(112)
    bodypk[:, C_EMB + 1] = embedding[1:, 1, :].reshape(112)
    bodypk[0:32, C_EMB + 2] = embedding[0].reshape(32)
    bodypk[0:32, C_EMB + 3:C_EMB + 35] = head_w

    base = {"bodypk": bodypk}
    in_maps = []
    for c in range(NCORES):
        dslice = np.ascontiguousarray(d32[c * BC:(c + 1) * BC].T)
        in_maps.append({**base, "dataT": dslice})
    return in_maps


def postprocess(results):
    # per-core out is (64, BC) transposed; unshard to (B, 2, 32)
    full = np.concatenate(
        [np.ascontiguousarray(results[c]["out"].T) for c in range(NCORES)],
        axis=0)
    return full.reshape(B, 2, 32)


_CACHE = {}


def kernel(data, embedding, head_w, body_w, **kw):
    nc = _CACHE.get("nc")
    if nc is None:
        nc = build_program()
        _CACHE["nc"] = nc
    in_maps = make_in_maps(data, embedding, head_w, body_w)
    res = run_bass_kernel_spmd(nc, in_maps, core_ids=list(range(NCORES)))
    return postprocess(res.results)
